# revision 4
# baseline (speedup 1.0000x reference)
"""Trainium2 Bass kernel for the torch-faithful MultiHeadAttention module.

Math (validated vs the jax reference):
  qkv = x @ W_qkv.T + b_qkv                    # [B, S, 3E]
  qkv.view(B, H, -1, 3*hd)  is a PLAIN reshape, so "head" h is really the
  sequence block s in [128h, 128h+128), and within a head the 2048 rows are
  s' = (s%128)*16 + j with j = f//192; q/k/v are column slices of each
  192-wide block j.
  score = q @ k.T / 8 ; softmax ; context ; out = context' @ W_out.T + b_out

Sharding (8 cores): data-parallel over batch (4 cores per batch element),
head-parallel within the group (4 heads per core).  Each core computes its
heads' attention entirely on-chip (flash style, no HBM score matrix) and a
partial out-projection over its 256 context columns; the host sums the 4
partials per batch element (a pure unshard/reduce step) and adds b_out.

v3 restructure (baseline 260us was flash-ACT-bound at exp N=1024 plus a
serial 51us proj phase):
  - scores row-packed: heads (2p, 2p+1) live on partition halves of qT/kT;
    two K=64 matmuls run concurrently in the PE array via tile_position
    auto-derive (score PE time halved)
  - exp units of N=2048/N=1024 (alternating 4-bank / 2-bank psum tiles,
    double-buffered by the alternation itself) cut ACT exp overhead
    146.6us -> ~127us
  - v is produced pre-transposed by direct x @ Wv^T matmuls (N=128,
    lhsT = xT head-block), killing the 32 PE transposes + DVE casts
  - W_qkv is repacked host-side into q|k blocks (DMA'd first) and v|v
    blocks (DMA'd later); flash starts right after the qk projection
  - chunk-major flash loop (window outer, pair inner); the out-projection
    of window c is woven into window c+1's units; v-projection batches are
    woven into the first two windows
"""

import numpy as np

import concourse.bass as bass
import concourse.mybir as mybir
import concourse.tile as tile
from concourse import bacc
from concourse.bass_utils import run_bass_kernel_spmd

B, S, E = 2, 2048, 1024
H, HD = 16, 64
NHL = 4   # heads per core
NP = 2    # head pairs per core
NJ = 16   # 192-wide j blocks (= kt key blocks per head)
P = 128
ET = E // P  # 8 contraction tiles of 128
W = 512   # query window
NW = S // W
F32 = mybir.dt.float32
BF16 = mybir.dt.bfloat16
EXP = mybir.ActivationFunctionType.Exp
IDENT = mybir.ActivationFunctionType.Identity

# per-window unit pattern: 2-kt units use a 4-bank psum tile (exp N=2048),
# 1-kt units a 2-bank tile (exp N=1024); the alternation double-buffers.
UNITS = [2, 1, 2, 1, 2, 1, 2, 1, 2, 1, 1]

_NC_CACHE = None
_LAST_RESULT = None  # BassKernelResults of the most recent run (for test harness)


def _emit(nc, tc, xT, wqk, wv, bqk, bvT, woutT, outp):
    import contextlib

    with contextlib.ExitStack() as ctx:
        ctx.enter_context(nc.allow_low_precision(reason="bf16 matmul operands"))
        const = ctx.enter_context(tc.tile_pool(name="const", bufs=1))
        ppool = ctx.enter_context(tc.tile_pool(name="probs", bufs=4))
        npool = ctx.enter_context(tc.tile_pool(name="norm", bufs=2))
        opool = ctx.enter_context(tc.tile_pool(name="osb", bufs=3))
        psum = ctx.enter_context(tc.tile_pool(name="ps", bufs=1, space="PSUM"))

        # ---- resident tiles -------------------------------------------------
        xT_sb = const.tile([P, ET, NHL * P], BF16, tag="xT")    # [128, 8, 512]
        wqk_sb = const.tile([P, NJ, ET, P], BF16, tag="wqk")    # 4.2MB
        wv_sb = const.tile([P, NJ // 2, ET, P], BF16, tag="wv")  # 2.1MB
        bqk_sb = const.tile([P, NJ], F32, tag="bqk")
        bv_sb = const.tile([P, NJ // 2, P], BF16, tag="bv")
        # q/k transposed, pair-interleaved: partitions 0:64 = head 2p dims,
        # 64:128 = head 2p+1 dims; free = (pair, q'' = j*128 + s)
        qT = const.tile([P, NP, S], BF16, tag="qT")
        kT = const.tile([P, NP, S], BF16, tag="kT")
        # v per head per j-block: [128 keys, 64 v dims + 1 ones col]
        vaug = const.tile([P, NHL, NJ, HD + 1], BF16, tag="vaug")
        # normalized context^T: K-tile p holds heads (2p, 2p+1) on halves
        ctxT = const.tile([P, NP, S], BF16, tag="ctxT")
        woutT_sb = const.tile([P, 2, E], BF16, tag="woutT")

        # ---- input DMA, ordered by consumption ------------------------------
        dmaq = [nc.gpsimd, nc.sync, nc.scalar]
        nc.sync.dma_start(out=bqk_sb, in_=bqk[:, :])
        for et in range(ET):
            dmaq[et % 3].dma_start(out=xT_sb[:, et, :], in_=xT[et, :, :])
        for j in range(NJ):
            dmaq[j % 3].dma_start(out=wqk_sb[:, j, :, :], in_=wqk[j, :, :, :])
        for jp in range(NJ // 2):
            dmaq[jp % 3].dma_start(out=wv_sb[:, jp, :, :], in_=wv[jp, :, :, :])
        for jp in range(NJ // 2):
            nc.gpsimd.dma_start(
                out=bv_sb[:, jp, :], in_=bvT[jp:jp + 1, :].to_broadcast([P, P])
            )
        nc.gpsimd.dma_start(
            out=woutT_sb, in_=woutT[:, :, :].rearrange("t p c -> p t c")
        )
        nc.vector.memset(vaug[:, :, :, HD:HD + 1], 1.0)

        # ---- psum borrow ring: big (4 banks) / small (2 banks) --------------
        def big_tile(name):
            return psum.tile([P, 2048], F32, tag="big", name=name)

        def small_tile(name):
            return psum.tile([P, 1024], F32, tag="small", name=name)

        # ---- qkv projection (q|k blocks), all 4 heads at N=512 --------------
        # psum block slices cycle big(4)/small(2): j 0-3 big, 4-5 small, ...
        def proj_block(j, ps_j):
            for et in range(ET):
                nc.tensor.matmul(
                    ps_j,
                    lhsT=wqk_sb[:, j, et, :],
                    rhs=xT_sb[:, et, :],
                    start=(et == 0),
                    stop=(et == ET - 1),
                )
            src = ps_j.rearrange("d (h s) -> d h s", s=P)
            for dst, rows in ((qT, (0, HD)), (kT, (HD, P))):
                bias = bqk_sb[rows[0]:rows[1], j:j + 1]
                # even heads -> partitions 0:64, odd -> 64:128
                nc.scalar.activation(
                    out=dst[0:HD, :, j * P:(j + 1) * P],
                    in_=src[rows[0]:rows[1], 0::2, :],
                    func=IDENT, bias=bias,
                )
                nc.vector.tensor_scalar_add(
                    out=dst[HD:P, :, j * P:(j + 1) * P],
                    in0=src[rows[0]:rows[1], 1::2, :],
                    scalar1=bias,
                )

        bj = 0
        while bj < NJ:
            if bj % 6 < 4:
                t = big_tile(f"pj{bj}")
                for q in range(4):
                    proj_block(bj + q, t[:, q * 512:(q + 1) * 512])
                bj += 4
            else:
                t = small_tile(f"pj{bj}")
                for q in range(2):
                    proj_block(bj + q, t[:, q * 512:(q + 1) * 512])
                bj += 2

        # ---- v projection, pre-transposed: one batch = (head, j-pair) ------
        def vt_batch(h, jp):
            vps = small_tile(f"v{h}_{jp}")[:, 0:P]
            for et in range(ET):
                nc.tensor.matmul(
                    vps,
                    lhsT=xT_sb[:, et, h * P:(h + 1) * P],
                    rhs=wv_sb[:, jp, et, :],
                    start=(et == 0),
                    stop=(et == ET - 1),
                )
            nc.vector.tensor_tensor(
                out=vaug[:, h, 2 * jp:2 * jp + 2, 0:HD],
                in0=vps.rearrange("s (j d) -> s j d", d=HD),
                in1=bv_sb[:, jp, :].rearrange("s (j d) -> s j d", d=HD),
                op=mybir.AluOpType.add,
            )

        # ---- norm machinery -------------------------------------------------
        # softmax denominators: DRAM reshape round-trip so the reciprocal
        # runs on 128 DVE lanes; the casting broadcast rides gpsimd.
        lscr = nc.dram_tensor("l_scratch", [NHL, S], F32).ap()
        rscratch = nc.dram_tensor("rinv_scratch", [NHL, S], F32).ap()
        pending = []  # (p, c, csb[2]) awaiting norm flush

        def finish(p, c, ps_ctx):
            q0 = c * W
            csbs = []
            for e in range(2):
                h = 2 * p + e
                l_sb = npool.tile([1, W], F32, tag="lrow", name="l_sb")
                nc.vector.tensor_copy(out=l_sb, in_=ps_ctx[e][HD:HD + 1, :])
                nc.sync.dma_start(out=lscr[h, q0:q0 + W], in_=l_sb)
                csb = npool.tile([HD, W], BF16, tag="csb", bufs=4, name="csb")
                nc.vector.tensor_copy(out=csb, in_=ps_ctx[e][0:HD, :])
                csbs.append(csb)
            l128 = npool.tile([P, 2 * W // P], F32, tag="l128", name="l128")
            nc.sync.dma_start(out=l128, in_=lscr[2 * p:2 * p + 2, q0:q0 + W])
            rinv = npool.tile([P, 2 * W // P], F32, tag="rinv", name="rinv")
            nc.vector.reciprocal(out=rinv, in_=l128)
            nc.sync.dma_start(out=rscratch[2 * p:2 * p + 2, q0:q0 + W], in_=rinv)
            pending.append((p, c, csbs))

        def flush_norm():
            p, c, csbs = pending.pop(0)
            q0 = c * W
            for e in range(2):
                h = 2 * p + e
                rb = npool.tile([HD, W], BF16, tag="rb", name="rb")
                nc.gpsimd.dma_start(
                    out=rb, in_=rscratch[h:h + 1, q0:q0 + W].to_broadcast([HD, W])
                )
                nc.vector.tensor_tensor(
                    out=ctxT[e * HD:(e + 1) * HD, p, q0:q0 + W],
                    in0=csbs[e],
                    in1=rb,
                    op=mybir.AluOpType.mult,
                )

        # ---- partial out-projection ----------------------------------------
        # out_part[s', f] = sum_d ctxT[d, s''] woutT[d, f], written to DRAM
        # with the s'' -> s' = 16 r + j permutation in the AP.
        out_view = outp.rearrange("(r six) f -> six r f", six=NJ)  # [16, 128, E]
        odma = [0]

        def outproj_st(st, tail=False):
            o_sb = opool.tile([P, E], BF16, tag="osb", name="o_sb")
            pos = small_tile(f"o{st}")
            for fc in range(2):
                for t in range(2):
                    nc.tensor.matmul(
                        pos[:, fc * 512:(fc + 1) * 512],
                        lhsT=ctxT[:, t, st * P:(st + 1) * P],
                        rhs=woutT_sb[:, t, fc * 512:(fc + 1) * 512],
                        start=(t == 0),
                        stop=(t == 1),
                    )
            nc.vector.tensor_copy(out=o_sb, in_=pos)
            engs = [nc.gpsimd, nc.sync, nc.scalar] if tail else [nc.gpsimd, nc.sync]
            engs[odma[0] % len(engs)].dma_start(out=out_view[st, :, :], in_=o_sb)
            odma[0] += 1

        # ---- flash window ---------------------------------------------------
        def flash_window(p, c, weave):
            q0 = c * W
            ps_ctx = [
                psum.tile([HD + 1, W], F32, tag="ctx", bufs=2, name=f"ctx{p}{c}{e}")
                for e in range(2)
            ]
            backlog = []

            def emit_ctx(item):
                pb, kts, offs = item
                for kt, off in zip(kts, offs):
                    for e in range(2):
                        nc.tensor.matmul(
                            ps_ctx[e],
                            lhsT=vaug[:, 2 * p + e, kt, :],
                            rhs=pb[:, off + e * W:off + (e + 1) * W],
                            start=(kt == 0),
                            stop=(kt == NJ - 1),
                        )

            kt = 0
            for ui, nkt in enumerate(UNITS):
                kts = list(range(kt, kt + nkt))
                kt += nkt
                if nkt == 2:
                    u = big_tile(f"u{p}{c}{ui}")
                else:
                    u = small_tile(f"u{p}{c}{ui}")
                for i, k in enumerate(kts):
                    nc.tensor.matmul(
                        u[:, i * 1024:i * 1024 + W],
                        lhsT=kT[0:HD, p, k * P:(k + 1) * P],
                        rhs=qT[0:HD, p, q0:q0 + W],
                        start=True, stop=True,
                    )
                    nc.tensor.matmul(
                        u[:, i * 1024 + W:(i + 1) * 1024],
                        lhsT=kT[HD:P, p, k * P:(k + 1) * P],
                        rhs=qT[HD:P, p, q0:q0 + W],
                        start=True, stop=True,
                    )
                pb = ppool.tile([P, 2048], BF16, tag="pT", name=f"pb{p}{c}{ui}")
                pb = pb[:, 0:nkt * 1024]
                nc.scalar.activation(out=pb, in_=u, func=EXP, scale=0.125)
                backlog.append((pb, kts, [i * 1024 for i in range(nkt)]))
                if len(backlog) > 2:
                    emit_ctx(backlog.pop(0))
                for fn in weave.get(ui, ()):
                    fn()
            while backlog:
                emit_ctx(backlog.pop(0))
            finish(p, c, ps_ctx)

        # ---- schedule -------------------------------------------------------
        for c in range(NW):
            for p in range(NP):
                weave = {}
                if c == 0:
                    # v projection batches: vaug[(2p, 2p+1), 2jp..2jp+1] must
                    # land before this window's ctx(kt=2jp) (unit ~jp*1.4+2)
                    for jp in range(NJ // 2):
                        weave.setdefault(jp, []).extend([
                            (lambda h=2 * p, q=jp: vt_batch(h, q)),
                            (lambda h=2 * p + 1, q=jp: vt_batch(h, q)),
                        ])
                else:
                    if p == 0:
                        # norm flush of (p1, c-1); (p0, c-1) flushed earlier
                        weave.setdefault(1, []).append(flush_norm)
                        weave.setdefault(4, []).append(flush_norm)
                        # out-projection of window c-1
                        for i, st in enumerate(range(4 * (c - 1), 4 * c)):
                            weave.setdefault(5 + i, []).append(
                                lambda s=st: outproj_st(s))
                flash_window(p, c, weave)
        while pending:
            flush_norm()
        for st in range(4 * (NW - 1), 4 * NW):
            outproj_st(st, tail=True)


def build_nc():
    nc = bacc.Bacc("TRN2", target_bir_lowering=False, debug=False, num_devices=8)
    xT = nc.declare_dram_parameter("xT", [ET, P, NHL * P], BF16, isOutput=False)
    wqk = nc.declare_dram_parameter("wqk", [NJ, P, ET, P], BF16, isOutput=False)
    wv = nc.declare_dram_parameter("wv", [NJ // 2, P, ET, P], BF16, isOutput=False)
    bqk = nc.declare_dram_parameter("bqk", [P, NJ], F32, isOutput=False)
    bvT = nc.declare_dram_parameter("bvT", [NJ // 2, P], F32, isOutput=False)
    woutT = nc.declare_dram_parameter("woutT", [2, P, E], BF16, isOutput=False)
    outp = nc.declare_dram_parameter("out_part", [S, E], BF16, isOutput=True)
    with tile.TileContext(nc) as tc:
        _emit(nc, tc, xT, wqk, wv, bqk, bvT, woutT, outp)
    nc.compile()
    return nc


def make_in_maps(x, W_qkv, b_qkv, W_out):
    import ml_dtypes
    bf16 = ml_dtypes.bfloat16
    x = np.asarray(x, np.float32)
    Wq = np.asarray(W_qkv, np.float32)          # [3072, 1024]
    b_qkv = np.asarray(b_qkv, np.float32)
    woutT = np.ascontiguousarray(np.asarray(W_out, np.float32).T)  # [1024c, 1024f]

    # q|k rows of block j: W_qkv rows [192j, 192j+128)
    wqk = np.stack([
        Wq[192 * j:192 * j + 128, :].T.reshape(ET, P, P).transpose(1, 0, 2)
        for j in range(NJ)
    ]).astype(bf16)                              # [16, p(e), et, c(f)]
    # v rows of j-pair jp: [v_{2jp} (64) | v_{2jp+1} (64)]
    wv = np.stack([
        np.concatenate([
            Wq[192 * (2 * jp) + 128:192 * (2 * jp) + 192, :],
            Wq[192 * (2 * jp + 1) + 128:192 * (2 * jp + 1) + 192, :],
        ]).T.reshape(ET, P, P).transpose(1, 0, 2)
        for jp in range(NJ // 2)
    ]).astype(bf16)                              # [8, p(e), et, c]
    bqk = np.stack([b_qkv[192 * j:192 * j + 128] for j in range(NJ)], axis=1)
    bqk = np.ascontiguousarray(bqk)              # [128, 16]
    bvT = np.stack([
        np.concatenate([
            b_qkv[192 * (2 * jp) + 128:192 * (2 * jp) + 192],
            b_qkv[192 * (2 * jp + 1) + 128:192 * (2 * jp + 1) + 192],
        ]) for jp in range(NJ // 2)
    ])                                           # [8, 128]
    bvT = np.ascontiguousarray(bvT.astype(np.float32))

    in_maps = []
    for core in range(8):
        b, g = divmod(core, 4)
        in_maps.append({
            "xT": np.ascontiguousarray(
                x[b, 512 * g:512 * (g + 1), :].T.reshape(ET, P, NHL * P)
            ).astype(bf16),
            "wqk": wqk,
            "wv": wv,
            "bqk": bqk,
            "bvT": bvT,
            "woutT": np.ascontiguousarray(
                woutT[256 * g:256 * (g + 1), :].reshape(2, P, E)
            ).astype(bf16),
        })
    return in_maps


def kernel(x, W_qkv, b_qkv, W_out, b_out):
    global _NC_CACHE, _LAST_RESULT
    if _NC_CACHE is None:
        _NC_CACHE = build_nc()
    in_maps = make_in_maps(x, W_qkv, b_qkv, W_out)
    _LAST_RESULT = run_bass_kernel_spmd(_NC_CACHE, in_maps, list(range(8)))
    res = _LAST_RESULT.results
    b_out = np.asarray(b_out, np.float32)
    out = np.empty((B, S, E), np.float32)
    for b in range(B):
        acc = np.asarray(res[4 * b]["out_part"], np.float32).copy()
        for g in range(1, 4):
            acc += np.asarray(res[4 * b + g]["out_part"], np.float32)
        out[b] = acc + b_out
    return out


# revision 7
# speedup vs baseline: 1.0884x; 1.0884x over previous
"""Trainium2 Bass kernel for the torch-faithful MultiHeadAttention module.

Math (validated vs the jax reference):
  qkv = x @ W_qkv.T + b_qkv                    # [B, S, 3E]
  qkv.view(B, H, -1, 3*hd)  is a PLAIN reshape, so "head" h is really the
  sequence block s in [128h, 128h+128), and within a head the 2048 rows are
  s' = (s%128)*16 + j with j = f//192; q/k/v are column slices of each
  192-wide block j.
  score = q @ k.T / 8 ; softmax ; context ; out = context' @ W_out.T + b_out

Sharding (8 cores): data-parallel over batch (4 cores per batch element),
head-parallel within the group (4 heads per core).  Each core computes its
heads' attention entirely on-chip (flash style, no HBM score matrix) and a
partial out-projection over its 256 context columns; the host sums the 4
partials per batch element (a pure unshard/reduce step) and adds b_out.

v3 restructure (baseline 260us was flash-ACT-bound at exp N=1024 plus a
serial 51us proj phase):
  - scores row-packed: heads (2p, 2p+1) live on partition halves of qT/kT;
    two K=64 matmuls run concurrently in the PE array via tile_position
    auto-derive (score PE time halved)
  - exp units of N=2048/N=1024 (alternating 4-bank / 2-bank psum tiles,
    double-buffered by the alternation itself) cut ACT exp overhead
    146.6us -> ~127us
  - v is produced pre-transposed by direct x @ Wv^T matmuls (N=128,
    lhsT = xT head-block), killing the 32 PE transposes + DVE casts
  - W_qkv is repacked host-side into q|k blocks (DMA'd first) and v|v
    blocks (DMA'd later); flash starts right after the qk projection
  - chunk-major flash loop (window outer, pair inner); the out-projection
    of window c is woven into window c+1's units; v-projection batches are
    woven into the first two windows
"""

import numpy as np

import concourse.bass as bass
import concourse.mybir as mybir
import concourse.tile as tile
from concourse import bacc
from concourse.bass_utils import run_bass_kernel_spmd

B, S, E = 2, 2048, 1024
H, HD = 16, 64
NHL = 4   # heads per core
NP = 2    # head pairs per core
NJ = 16   # 192-wide j blocks (= kt key blocks per head)
P = 128
ET = E // P  # 8 contraction tiles of 128
W = 512   # query window
NW = S // W
F32 = mybir.dt.float32
BF16 = mybir.dt.bfloat16
EXP = mybir.ActivationFunctionType.Exp
IDENT = mybir.ActivationFunctionType.Identity

# per-window unit pattern: 2-kt units use a 4-bank psum tile (exp N=2048),
# 1-kt units a 2-bank tile (exp N=1024); the alternation double-buffers.
UNITS = [2, 1, 2, 1, 2, 1, 2, 1, 2, 1, 1]

_NC_CACHE = None
_LAST_RESULT = None  # BassKernelResults of the most recent run (for test harness)


def _emit(nc, tc, xT, wqk, wv, bqk, bvT, woutT, outp):
    import contextlib

    with contextlib.ExitStack() as ctx:
        ctx.enter_context(nc.allow_low_precision(reason="bf16 matmul operands"))
        const = ctx.enter_context(tc.tile_pool(name="const", bufs=1))
        ppool = ctx.enter_context(tc.tile_pool(name="probs", bufs=4))
        npool = ctx.enter_context(tc.tile_pool(name="norm", bufs=2))
        opool = ctx.enter_context(tc.tile_pool(name="osb", bufs=3))
        psum = ctx.enter_context(tc.tile_pool(name="ps", bufs=1, space="PSUM"))

        # ---- resident tiles -------------------------------------------------
        xT_sb = const.tile([P, ET, NHL * P], BF16, tag="xT")    # [128, 8, 512]
        wqk_sb = const.tile([P, NJ, ET, P], BF16, tag="wqk")    # 4.2MB
        wv_sb = const.tile([P, NJ // 2, ET, P], BF16, tag="wv")  # 2.1MB
        bqk_sb = const.tile([P, NJ], F32, tag="bqk")
        bv_sb = const.tile([P, NJ // 2, P], BF16, tag="bv")
        # q/k transposed, pair-interleaved: partitions 0:64 = head 2p dims
        # (slot 0 of the dual-slot free axis), 64:128 = head 2p+1 dims
        # (slot 1).  Separate byte ranges for the two partition halves keep
        # the (partition-blind) dep tracker from serializing their writers.
        qT = const.tile([P, NP, NJ, 2, P], BF16, tag="qT")
        kT = const.tile([P, NP, NJ, 2, P], BF16, tag="kT")
        # v per head per j-block: [128 keys, 64 v dims + 1 ones col]
        vaug = const.tile([P, NHL, NJ, HD + 1], BF16, tag="vaug")
        # normalized context^T: K-tile p holds heads (2p, 2p+1) on halves
        ctxT = const.tile([P, NP, S], BF16, tag="ctxT")
        woutT_sb = const.tile([P, 2, E], BF16, tag="woutT")

        # ---- input DMA, ordered by consumption ------------------------------
        dmaq = [nc.gpsimd, nc.sync, nc.scalar]
        nc.sync.dma_start(out=bqk_sb, in_=bqk[:, :])
        for et in range(ET):
            dmaq[et % 3].dma_start(out=xT_sb[:, et, :], in_=xT[et, :, :])
        for j in range(NJ):
            dmaq[j % 3].dma_start(out=wqk_sb[:, j, :, :], in_=wqk[j, :, :, :])
        for jp in range(NJ // 2):
            dmaq[jp % 3].dma_start(out=wv_sb[:, jp, :, :], in_=wv[jp, :, :, :])
        for jp in range(NJ // 2):
            nc.gpsimd.dma_start(
                out=bv_sb[:, jp, :], in_=bvT[jp:jp + 1, :].to_broadcast([P, P])
            )
        nc.gpsimd.dma_start(
            out=woutT_sb, in_=woutT[:, :, :].rearrange("t p c -> p t c")
        )
        nc.vector.memset(vaug[:, :, :, HD:HD + 1], 1.0)

        # ---- psum borrow ring: big (4 banks) / small (2 banks) --------------
        def big_tile(name):
            return psum.tile([P, 2048], F32, tag="big", name=name)

        def small_tile(name):
            return psum.tile([P, 1024], F32, tag="small", name=name)

        # ---- qkv projection (q|k blocks), all 4 heads at N=512 --------------
        # psum block slices cycle big(4)/small(2): j 0-3 big, 4-5 small, ...
        def proj_block(j, ps_j):
            for et in range(ET):
                nc.tensor.matmul(
                    ps_j,
                    lhsT=wqk_sb[:, j, et, :],
                    rhs=xT_sb[:, et, :],
                    start=(et == 0),
                    stop=(et == ET - 1),
                )
            src = ps_j.rearrange("d (h s) -> d h s", s=P)
            for dst, rows in ((qT, (0, HD)), (kT, (HD, P))):
                bias = bqk_sb[rows[0]:rows[1], j:j + 1]
                # even heads -> partitions 0:64 slot 0, odd -> 64:128 slot 1
                nc.scalar.activation(
                    out=dst[0:HD, :, j, 0, :],
                    in_=src[rows[0]:rows[1], 0::2, :],
                    func=IDENT, bias=bias,
                )
                nc.vector.tensor_scalar_add(
                    out=dst[HD:P, :, j, 1, :],
                    in0=src[rows[0]:rows[1], 1::2, :],
                    scalar1=bias,
                )

        # 2 blocks per psum tile, alternating tags so each tile's recycle
        # WAR is hidden behind the other tag's chains
        for bj in range(0, NJ, 2):
            t = big_tile(f"pj{bj}") if bj % 4 == 0 else small_tile(f"pj{bj}")
            for q in range(2):
                proj_block(bj + q, t[:, q * 512:(q + 1) * 512])

        # ---- v projection, pre-transposed: one batch = (head, j-pair) ------
        def vt_batch(h, jp):
            vps = small_tile(f"v{h}_{jp}")[:, 0:P]
            for et in range(ET):
                nc.tensor.matmul(
                    vps,
                    lhsT=xT_sb[:, et, h * P:(h + 1) * P],
                    rhs=wv_sb[:, jp, et, :],
                    start=(et == 0),
                    stop=(et == ET - 1),
                )
            nc.vector.tensor_tensor(
                out=vaug[:, h, 2 * jp:2 * jp + 2, 0:HD],
                in0=vps.rearrange("s (j d) -> s j d", d=HD),
                in1=bv_sb[:, jp, :].rearrange("s (j d) -> s j d", d=HD),
                op=mybir.AluOpType.add,
            )

        # ---- norm machinery -------------------------------------------------
        # softmax denominators: DRAM reshape round-trip so the reciprocal
        # runs on 128 DVE lanes; the casting broadcast rides gpsimd.
        lscr = nc.dram_tensor("l_scratch", [NHL, S], F32).ap()
        rscratch = nc.dram_tensor("rinv_scratch", [NHL, S], F32).ap()
        pending = []  # (p, c, csb[2]) awaiting norm flush

        def finish(p, c, ps_ctx):
            q0 = c * W
            csbs = []
            for e in range(2):
                h = 2 * p + e
                l_sb = npool.tile([1, W], F32, tag="lrow", name="l_sb")
                nc.vector.tensor_copy(out=l_sb, in_=ps_ctx[e][HD:HD + 1, :])
                nc.sync.dma_start(out=lscr[h, q0:q0 + W], in_=l_sb)
                csb = npool.tile([HD, W], BF16, tag="csb", bufs=4, name="csb")
                nc.vector.tensor_copy(out=csb, in_=ps_ctx[e][0:HD, :])
                csbs.append(csb)
            l128 = npool.tile([P, 2 * W // P], F32, tag="l128", name="l128")
            nc.sync.dma_start(out=l128, in_=lscr[2 * p:2 * p + 2, q0:q0 + W])
            rinv = npool.tile([P, 2 * W // P], F32, tag="rinv", name="rinv")
            nc.vector.reciprocal(out=rinv, in_=l128)
            nc.sync.dma_start(out=rscratch[2 * p:2 * p + 2, q0:q0 + W], in_=rinv)
            pending.append((p, c, csbs))

        def flush_norm():
            p, c, csbs = pending.pop(0)
            q0 = c * W
            for e in range(2):
                h = 2 * p + e
                rb = npool.tile([HD, W], BF16, tag="rb", name="rb")
                nc.gpsimd.dma_start(
                    out=rb, in_=rscratch[h:h + 1, q0:q0 + W].to_broadcast([HD, W])
                )
                nc.vector.tensor_tensor(
                    out=ctxT[e * HD:(e + 1) * HD, p, q0:q0 + W],
                    in0=csbs[e],
                    in1=rb,
                    op=mybir.AluOpType.mult,
                )

        # ---- partial out-projection ----------------------------------------
        # out_part[s', f] = sum_d ctxT[d, s''] woutT[d, f], written to DRAM
        # with the s'' -> s' = 16 r + j permutation in the AP.
        out_view = outp.rearrange("(r six) f -> six r f", six=NJ)  # [16, 128, E]
        odma = [0]

        def outproj_st(st, tail=False):
            o_sb = opool.tile([P, E], BF16, tag="osb", name="o_sb")
            pos = small_tile(f"o{st}")
            for fc in range(2):
                for t in range(2):
                    nc.tensor.matmul(
                        pos[:, fc * 512:(fc + 1) * 512],
                        lhsT=ctxT[:, t, st * P:(st + 1) * P],
                        rhs=woutT_sb[:, t, fc * 512:(fc + 1) * 512],
                        start=(t == 0),
                        stop=(t == 1),
                    )
            nc.vector.tensor_copy(out=o_sb, in_=pos)
            engs = [nc.gpsimd, nc.sync, nc.scalar] if tail else [nc.gpsimd, nc.sync]
            engs[odma[0] % len(engs)].dma_start(out=out_view[st, :, :], in_=o_sb)
            odma[0] += 1

        # ---- flash: one flat pipeline across all (c, p) windows -------------
        # Unit u's scores+exp are emitted immediately; its ctx matmuls lag
        # 2 units so the next window's scores never sit behind a ctx tail
        # in the PE FIFO.  finish() is emitted right after a window's last
        # ctx.  Weave items (v-proj batches, norm flushes, out-projections)
        # attach to global unit indices.
        weave = {}
        NU = len(UNITS)
        for wi in range(2):  # (c0,p0) -> units 0..10, (c0,p1) -> 11..21
            for jp in range(NJ // 2):
                weave.setdefault(wi * NU + jp, []).extend([
                    (lambda h=2 * wi, q=jp: vt_batch(h, q)),
                    (lambda h=2 * wi + 1, q=jp: vt_batch(h, q)),
                ])
        for c in range(1, NW):
            base = 2 * c * NU  # window (c, p0)
            weave.setdefault(base + 1, []).append(flush_norm)
            weave.setdefault(base + 4, []).append(flush_norm)
            for i, st in enumerate(range(4 * (c - 1), 4 * c)):
                weave.setdefault(base + 5 + i, []).append(
                    lambda s=st: outproj_st(s))

        backlog = []  # (pb, kts, offs, p, ps_ctx, last)

        def emit_ctx(item):
            pb, kts, offs, p, ps_ctx, last = item
            for kt, off in zip(kts, offs):
                for e in range(2):
                    nc.tensor.matmul(
                        ps_ctx[e],
                        lhsT=vaug[:, 2 * p + e, kt, :],
                        rhs=pb[:, off + e * W:off + (e + 1) * W],
                        start=(kt == 0),
                        stop=(kt == NJ - 1),
                    )
            if last:
                finish(last[0], last[1], ps_ctx)

        gu = 0
        for c in range(NW):
            for p in range(NP):
                ps_ctx = [
                    psum.tile([HD + 1, W], F32, tag="ctx", bufs=2,
                              name=f"ctx{p}{c}{e}")
                    for e in range(2)
                ]
                kt = 0
                for ui, nkt in enumerate(UNITS):
                    kts = list(range(kt, kt + nkt))
                    kt += nkt
                    if nkt == 2:
                        u = big_tile(f"u{p}{c}{ui}")
                    else:
                        u = small_tile(f"u{p}{c}{ui}")
                    for i, k in enumerate(kts):
                        nc.tensor.matmul(
                            u[:, i * 1024:i * 1024 + W],
                            lhsT=kT[0:HD, p, k, 0, :],
                            rhs=qT[0:HD, p, 4 * c:4 * c + 4, 0, :],
                            start=True, stop=True,
                        )
                        nc.tensor.matmul(
                            u[:, i * 1024 + W:(i + 1) * 1024],
                            lhsT=kT[HD:P, p, k, 1, :],
                            rhs=qT[HD:P, p, 4 * c:4 * c + 4, 1, :],
                            start=True, stop=True,
                        )
                    pb = ppool.tile([P, 2048], BF16, tag="pT", name=f"pb{p}{c}{ui}")
                    pb = pb[:, 0:nkt * 1024]
                    nc.scalar.activation(out=pb, in_=u, func=EXP, scale=0.125)
                    backlog.append((
                        pb, kts, [i * 1024 for i in range(nkt)], p, ps_ctx,
                        (p, c) if ui == NU - 1 else None,
                    ))
                    if len(backlog) > 2:
                        emit_ctx(backlog.pop(0))
                    for fn in weave.get(gu, ()):
                        fn()
                    gu += 1
        while backlog:
            emit_ctx(backlog.pop(0))
        while pending:
            flush_norm()
        for st in range(4 * (NW - 1), 4 * NW):
            outproj_st(st, tail=True)


def build_nc():
    nc = bacc.Bacc("TRN2", target_bir_lowering=False, debug=False, num_devices=8)
    xT = nc.declare_dram_parameter("xT", [ET, P, NHL * P], BF16, isOutput=False)
    wqk = nc.declare_dram_parameter("wqk", [NJ, P, ET, P], BF16, isOutput=False)
    wv = nc.declare_dram_parameter("wv", [NJ // 2, P, ET, P], BF16, isOutput=False)
    bqk = nc.declare_dram_parameter("bqk", [P, NJ], F32, isOutput=False)
    bvT = nc.declare_dram_parameter("bvT", [NJ // 2, P], F32, isOutput=False)
    woutT = nc.declare_dram_parameter("woutT", [2, P, E], BF16, isOutput=False)
    outp = nc.declare_dram_parameter("out_part", [S, E], BF16, isOutput=True)
    with tile.TileContext(nc) as tc:
        _emit(nc, tc, xT, wqk, wv, bqk, bvT, woutT, outp)
    nc.compile()
    return nc


def make_in_maps(x, W_qkv, b_qkv, W_out):
    import ml_dtypes
    bf16 = ml_dtypes.bfloat16
    x = np.asarray(x, np.float32)
    Wq = np.asarray(W_qkv, np.float32)          # [3072, 1024]
    b_qkv = np.asarray(b_qkv, np.float32)
    woutT = np.ascontiguousarray(np.asarray(W_out, np.float32).T)  # [1024c, 1024f]

    # q|k rows of block j: W_qkv rows [192j, 192j+128)
    wqk = np.stack([
        Wq[192 * j:192 * j + 128, :].T.reshape(ET, P, P).transpose(1, 0, 2)
        for j in range(NJ)
    ]).astype(bf16)                              # [16, p(e), et, c(f)]
    # v rows of j-pair jp: [v_{2jp} (64) | v_{2jp+1} (64)]
    wv = np.stack([
        np.concatenate([
            Wq[192 * (2 * jp) + 128:192 * (2 * jp) + 192, :],
            Wq[192 * (2 * jp + 1) + 128:192 * (2 * jp + 1) + 192, :],
        ]).T.reshape(ET, P, P).transpose(1, 0, 2)
        for jp in range(NJ // 2)
    ]).astype(bf16)                              # [8, p(e), et, c]
    bqk = np.stack([b_qkv[192 * j:192 * j + 128] for j in range(NJ)], axis=1)
    bqk = np.ascontiguousarray(bqk)              # [128, 16]
    bvT = np.stack([
        np.concatenate([
            b_qkv[192 * (2 * jp) + 128:192 * (2 * jp) + 192],
            b_qkv[192 * (2 * jp + 1) + 128:192 * (2 * jp + 1) + 192],
        ]) for jp in range(NJ // 2)
    ])                                           # [8, 128]
    bvT = np.ascontiguousarray(bvT.astype(np.float32))

    in_maps = []
    for core in range(8):
        b, g = divmod(core, 4)
        in_maps.append({
            "xT": np.ascontiguousarray(
                x[b, 512 * g:512 * (g + 1), :].T.reshape(ET, P, NHL * P)
            ).astype(bf16),
            "wqk": wqk,
            "wv": wv,
            "bqk": bqk,
            "bvT": bvT,
            "woutT": np.ascontiguousarray(
                woutT[256 * g:256 * (g + 1), :].reshape(2, P, E)
            ).astype(bf16),
        })
    return in_maps


def kernel(x, W_qkv, b_qkv, W_out, b_out):
    global _NC_CACHE, _LAST_RESULT
    if _NC_CACHE is None:
        _NC_CACHE = build_nc()
    in_maps = make_in_maps(x, W_qkv, b_qkv, W_out)
    _LAST_RESULT = run_bass_kernel_spmd(_NC_CACHE, in_maps, list(range(8)))
    res = _LAST_RESULT.results
    b_out = np.asarray(b_out, np.float32)
    out = np.empty((B, S, E), np.float32)
    for b in range(B):
        acc = np.asarray(res[4 * b]["out_part"], np.float32).copy()
        for g in range(1, 4):
            acc += np.asarray(res[4 * b + g]["out_part"], np.float32)
        out[b] = acc + b_out
    return out
